# revision 8
# baseline (speedup 1.0000x reference)
"""Fused single-head attention (QKV projection + softmax(QK^T/8) @ V) on 8
Trainium2 NeuronCores.

Problem: x [4, 2048, 1024] f32, kernel [3, 1024, 1024] f32 ->
         out [4, 2048, 1024] f32.

Sharding: 8 cores = 4 batches x 2 query-halves. Each core computes K^T and V
for its whole batch (redundantly with its pair core) plus Q for its query
half, then attention for its 1024 queries. No collectives.

All matmuls run in float32r (4x the fp32 PE rate, ~1e-4 relative error).
Every DMA is a fully contiguous copy: the host pre-arranges x^T and the
weights into the exact SBUF tile layouts (per-chunk blocks), and the output
is written as contiguous [16,128,512] blocks the host reassembles.

Per-core dataflow:
  KT[o,k] = sum_d Wk[d,o] * x[k,d]        (lhsT=Wk, rhs=x^T)
  V[k,o]  = sum_d x[k,d] * Wv[d,o]        (lhsT=x^T, rhs=Wv)
  QT[o,q] = sum_d Wq[d,o] * x[q,d]        (lhsT=Wq, rhs=x^T)
  S^T[k,q] = sum_o KT[o,k] * QT[o,q]      (lhsT=KT, rhs=QT)  -> PSUM
  E[k,q] = exp(S^T * 1/8)                 (ACT, no max-subtraction: |S/8|<~10)
  denom[q] = sum_k E[k,q]                 (lhsT=E, rhs=ones[128,2])
  out[q,o] = (sum_k E[k,q] * V[k,o]) / denom[q]
The transposed-scores layout needs no on-chip transposes anywhere and the
softmax needs no vector-engine reductions.
"""

import numpy as np
from contextlib import ExitStack

import concourse.bacc as bacc
import concourse.mybir as mybir
import concourse.tile as tile
from concourse.bass_utils import run_bass_kernel_spmd

F32 = mybir.dt.float32
F32R = mybir.dt.float32r
EXP = mybir.ActivationFunctionType.Exp

B, S, D, DO = 4, 2048, 1024, 1024
QH = S // 2        # queries per core
DC = D // 128      # contraction chunks for projections
OC = DO // 128     # output-dim 128-chunks
KC = S // 128      # key 128-chunks
SS = S // 512      # 512-wide s-chunks
SCALE = 1.0 / 8.0  # 1/sqrt(64) hardcoded in the reference


def _attention_core(tc):
    nc = tc.nc
    # Inputs pre-arranged by the host so every DMA below is one contiguous run.
    xt_d = nc.dram_tensor("xt", [SS, 128, DC, 512], F32R,
                          kind="ExternalInput").ap()
    xtq_d = nc.dram_tensor("xtq", [128, DC, QH], F32R, kind="ExternalInput").ap()
    wq_d = nc.dram_tensor("wq", [OC, 128, DC, 128], F32R,
                          kind="ExternalInput").ap()
    wk_d = nc.dram_tensor("wk", [128, DC, DO], F32R, kind="ExternalInput").ap()
    wv_d = nc.dram_tensor("wv", [128, DC, DO], F32R, kind="ExternalInput").ap()
    # out block idx = qb*8 + qs*2 + oh -> rows qb*512+qs*128, cols oh*512
    out_d = nc.dram_tensor("out", [16, 128, 512], F32, kind="ExternalOutput").ap()

    with ExitStack() as ctx:
        # Persistent tiles allocated phase-by-phase (stack allocator):
        # phase 1 peaks at KT+w+x = 128KB/part, phase 2 at KT+V+w+x = 192,
        # phase 3 at KT+V+QT+woc+xq = 200, phase 4 at KT+V+QT+exp+out ~ 202.
        pKT = ctx.enter_context(tc.tile_pool(name="pKT", bufs=1))
        KT = pKT.tile([128, OC, S], F32R, tag="KT")        # 64KB/part

        # ---- Phase 1: KT[o, k-chunk] ----
        with ExitStack() as pc:
            wpool = pc.enter_context(tc.tile_pool(name="wpool", bufs=1))
            xpool = pc.enter_context(tc.tile_pool(name="xpool", bufs=2))
            psum = pc.enter_context(tc.tile_pool(name="psum1", bufs=4, space="PSUM"))
            wkt = wpool.tile([128, DC, DO], F32R, tag="w")
            nc.sync.dma_start(wkt, wk_d)
            for ss in range(SS):
                xc = xpool.tile([128, DC, 512], F32R, tag="xt")
                nc.sync.dma_start(xc, xt_d[ss])
                for oc in range(OC):
                    ps = psum.tile([128, 512], F32, tag="ps")
                    for dc in range(DC):
                        nc.tensor.matmul(
                            ps, wkt[:, dc, oc * 128:(oc + 1) * 128], xc[:, dc],
                            start=(dc == 0), stop=(dc == DC - 1),
                        )
                    nc.vector.tensor_copy(
                        out=KT[:, oc, ss * 512:(ss + 1) * 512], in_=ps)

        # ---- Phase 2: V[k-chunk, o] ----
        pV = ctx.enter_context(tc.tile_pool(name="pV", bufs=1))
        V = pV.tile([128, KC, DO], F32R, tag="V")          # 64KB/part
        with ExitStack() as pc:
            wpool = pc.enter_context(tc.tile_pool(name="wpool2", bufs=1))
            xpool = pc.enter_context(tc.tile_pool(name="xpool2", bufs=2))
            psum = pc.enter_context(tc.tile_pool(name="psum2", bufs=4, space="PSUM"))
            wvt = wpool.tile([128, DC, DO], F32R, tag="w")
            nc.sync.dma_start(wvt, wv_d)
            for ss in range(SS):
                xc = xpool.tile([128, DC, 512], F32R, tag="xt")
                nc.sync.dma_start(xc, xt_d[ss])
                for ks in range(4):
                    for oh in range(DO // 512):
                        ps = psum.tile([128, 512], F32, tag="ps")
                        for dc in range(DC):
                            nc.tensor.matmul(
                                ps,
                                xc[:, dc, ks * 128:(ks + 1) * 128],
                                wvt[:, dc, oh * 512:(oh + 1) * 512],
                                start=(dc == 0), stop=(dc == DC - 1),
                            )
                        nc.vector.tensor_copy(
                            out=V[:, ss * 4 + ks, oh * 512:(oh + 1) * 512], in_=ps)

        # ---- Phase 3: QT[o, q] (wq streamed per o-chunk, x^T_q resident) ----
        pQT = ctx.enter_context(tc.tile_pool(name="pQT", bufs=1))
        QT = pQT.tile([128, OC, QH], F32R, tag="QT")       # 32KB/part
        with ExitStack() as pc:
            wpool = pc.enter_context(tc.tile_pool(name="wpool3", bufs=2))
            xpool = pc.enter_context(tc.tile_pool(name="xpool3", bufs=1))
            psum = pc.enter_context(tc.tile_pool(name="psum3", bufs=4, space="PSUM"))
            xq = xpool.tile([128, DC, QH], F32R, tag="xq")
            nc.sync.dma_start(xq, xtq_d)
            for oc in range(OC):
                woc = wpool.tile([128, DC, 128], F32R, tag="woc")
                nc.sync.dma_start(woc, wq_d[oc])
                for qs in range(QH // 512):
                    ps = psum.tile([128, 512], F32, tag="ps")
                    for dc in range(DC):
                        nc.tensor.matmul(
                            ps, woc[:, dc], xq[:, dc, qs * 512:(qs + 1) * 512],
                            start=(dc == 0), stop=(dc == DC - 1),
                        )
                    nc.vector.tensor_copy(
                        out=QT[:, oc, qs * 512:(qs + 1) * 512], in_=ps)

        # ---- Phase 4: attention ----
        with ExitStack() as pc:
            expp = pc.enter_context(tc.tile_pool(name="expp", bufs=1))
            cpool = pc.enter_context(tc.tile_pool(name="cpool", bufs=1))
            opool = pc.enter_context(tc.tile_pool(name="opool", bufs=4))
            rpool = pc.enter_context(tc.tile_pool(name="rpool", bufs=4))
            spsum = pc.enter_context(tc.tile_pool(name="spsum", bufs=4, space="PSUM"))
            dpsum = pc.enter_context(tc.tile_pool(name="dpsum", bufs=2, space="PSUM"))
            apsum = pc.enter_context(tc.tile_pool(name="apsum", bufs=2, space="PSUM"))

            ones_f32 = cpool.tile([128, 2], F32, tag="ones")
            nc.vector.memset(ones_f32, 1.0)
            ones = ones_f32.bitcast(F32R)

            for qb in range(QH // 512):
                qb_sl = slice(qb * 512, (qb + 1) * 512)
                exps = []
                for kc in range(KC):
                    ps = spsum.tile([128, 512], F32, tag="sp")
                    for oc in range(OC):
                        nc.tensor.matmul(
                            ps,
                            KT[:, oc, kc * 128:(kc + 1) * 128],
                            QT[:, oc, qb_sl],
                            start=(oc == 0), stop=(oc == OC - 1),
                        )
                    e = expp.tile([128, 512], F32R, tag=f"exp{kc}")
                    nc.scalar.activation(e, ps, EXP, scale=SCALE)
                    exps.append(e)
                for qs in range(4):
                    q_sl = slice(qs * 128, (qs + 1) * 128)
                    dps = dpsum.tile([128, 2], F32, tag="dp")
                    for kc in range(KC):
                        nc.tensor.matmul(
                            dps, exps[kc][:, q_sl], ones,
                            start=(kc == 0), stop=(kc == KC - 1),
                        )
                    rec = rpool.tile([128, 1], F32, tag="rec")
                    nc.vector.reciprocal(rec, dps[:, 0:1])
                    for oh in range(DO // 512):
                        aps = apsum.tile([128, 512], F32, tag="ap")
                        for kc in range(KC):
                            nc.tensor.matmul(
                                aps, exps[kc][:, q_sl],
                                V[:, kc, oh * 512:(oh + 1) * 512],
                                start=(kc == 0), stop=(kc == KC - 1),
                            )
                        ot = opool.tile([128, 512], F32, tag="ot")
                        nc.vector.tensor_scalar_mul(ot, aps, rec)
                        nc.sync.dma_start(out_d[qb * 8 + qs * 2 + oh], ot)


_NC_CACHE = None


def build_nc():
    global _NC_CACHE
    if _NC_CACHE is None:
        nc = bacc.Bacc("TRN2", target_bir_lowering=False, debug=False,
                       num_devices=8)
        with tile.TileContext(nc) as tc:
            _attention_core(tc)
        nc.compile()
        _NC_CACHE = nc
    return _NC_CACHE


def _prep_dxT(x2d):
    """[rows, 1024] -> [128, DC, rows]: t[p, dc, r] = x2d[r, dc*128+p]."""
    return np.ascontiguousarray(
        x2d.T.reshape(DC, 128, x2d.shape[0]).transpose(1, 0, 2))


def make_in_maps(x, w):
    # w_sb[p, dc, o] = w[i][dc*128+p, o]
    w_prep = [
        np.ascontiguousarray(w[i].reshape(DC, 128, DO).transpose(1, 0, 2))
        for i in range(3)
    ]
    # wq pre-chunked per o-chunk: [OC, 128, DC, 128]
    wq_c = np.ascontiguousarray(
        w_prep[0].reshape(128, DC, OC, 128).transpose(2, 0, 1, 3))
    in_maps = []
    for c in range(8):
        b, h = c // 2, c % 2
        xt = _prep_dxT(x[b])                       # [128, DC, S]
        # pre-chunked per 512-wide s block: [SS, 128, DC, 512]
        xt_c = np.ascontiguousarray(
            xt.reshape(128, DC, SS, 512).transpose(2, 0, 1, 3))
        xtq = np.ascontiguousarray(xt[:, :, h * QH:(h + 1) * QH])
        in_maps.append({
            "xt": xt_c, "xtq": xtq,
            "wq": wq_c, "wk": w_prep[1], "wv": w_prep[2],
        })
    return in_maps


def assemble_out(res_list):
    out = np.empty((B, S, DO), dtype=np.float32)
    for c in range(8):
        b, h = c // 2, c % 2
        blk = res_list[c]  # [16, 128, 512]
        core = blk.reshape(2, 4, 2, 128, 512).transpose(0, 1, 3, 2, 4)
        out[b, h * QH:(h + 1) * QH, :] = core.reshape(QH, DO)
    return out


def kernel(x, **rest):
    w = rest["kernel"]
    x = np.asarray(x, dtype=np.float32)
    w = np.asarray(w, dtype=np.float32)
    nc = build_nc()
    in_maps = make_in_maps(x, w)
    res = run_bass_kernel_spmd(nc, in_maps, list(range(8)))
    return assemble_out([res.results[c]["out"] for c in range(8)])
